# revision 32
# baseline (speedup 1.0000x reference)
"""Single attention head (B=8, S=2048, D_IN=1024, D_OUT=64) on 8 TRN2 NeuronCores.

Strategy: pure data-parallel over batch -- core b computes batch element b's
full attention head. No collectives.

v9 -- the v3 baseline schedule (exp-paced, qc-major, deferred ctx pops)
with four independently-verified upgrades:
  - ~4.5us of junk matmuls on a memset tile (no DMA dependency) pull the
    PE HAM clock-gate through the SHORT window to 8/8 while the first
    input DMAs are still in flight (first-chunk DMA latency is ~5us);
    v3's warmups waited on the consts DMA, so all of phase A ran at the
    1.2GHz cold clock.
  - The consts DMA is split hot (wkq/wqk/misc -- everything phase A
    needs) / cold (wv/identities, first needed by the phase-B V units),
    and seqf8[0] lands as c0 / c1 / c2c3 chunks, so the first projection
    starts as early as the DMA pipe allows.  Host layouts are
    partition-outermost so any [sj, c-range] slice is one
    contiguous-per-partition descriptor.  The scalar HWDGE queue gets
    only early never-blocking issues (a credit-blocked issue instruction
    stalls the exp stream behind it in ScalarE's FIFO).
  - finalize runs in fp16 (PE transpose w/ fp16 identity): half the f32
    transpose cycles and 2x DVE rate on the psum drain.  |num| <~ 1e3,
    den ~ 1.7e3: well inside fp16 range, 4.9e-4 rel quantization.
  - The very last exp (qc3, pair 7) is split into two half-tiles and
    qc3's output DMA into two chunks, shortening the serial tail after
    the final exp by ~1us.
Per-core dataflow otherwise identical to v3 (see its docstring): fp8
DoubleRow K/Q projections with x32-scaled stacked weights, score pairs
co-run on disjoint PE row groups, one exp per [128,1024] pair tile, mask
applied via zeroed V rows + keep-column denominator, ctx accumulated as
ctxT[65, q] with deferred pops as PE filler.
"""

import numpy as np
import ml_dtypes

import concourse.bass as bass  # noqa: F401
import concourse.mybir as mybir
import concourse.tile as tile
from concourse import bacc
from concourse.bass_utils import run_bass_kernel_spmd

B, S, D, F = 8, 2048, 1024, 64
NCORES = 8
BF = mybir.dt.bfloat16
F16 = mybir.dt.float16
F8 = mybir.dt.float8e4
F32 = mybir.dt.float32
SCALE = 1.0 / (1024.0 * float(np.sqrt(np.float32(S))))
SC = 512
NSJ = S // SC
KCH = S // 128
DCH = D // 128
DR = D // 256
HOT_B = 2124  # wkq 1024 | wqk 1024 | misc 76
COLD_B = 1410  # wv 1024 | identb 256 | identh 130


def _emit(nc):
    seqf8_d = nc.declare_dram_parameter("seqf8", [128, NSJ, DR, 2, SC], F8, isOutput=False)
    seqb_d = nc.declare_dram_parameter("seqb", [128, NSJ, DCH, SC], BF, isOutput=False)
    ch_d = nc.declare_dram_parameter("ch", [128, HOT_B], mybir.dt.uint8, isOutput=False)
    cc_d = nc.declare_dram_parameter("cc", [128, COLD_B], mybir.dt.uint8, isOutput=False)
    out_d = nc.declare_dram_parameter("out", [S, F], F32, isOutput=True)

    with tile.TileContext(nc) as tc:
        _body(nc, tc, seqf8_d, seqb_d, ch_d, cc_d, out_d)
    nc.compile()


def _body(nc, tc, seqf8_d, seqb_d, ch_d, cc_d, out_d):
    from contextlib import ExitStack

    with ExitStack() as ctx:
        const = ctx.enter_context(tc.tile_pool(name="const", bufs=1))
        big = ctx.enter_context(tc.tile_pool(name="big", bufs=1))
        sbw = ctx.enter_context(tc.tile_pool(name="sbw", bufs=1))
        ps = ctx.enter_context(tc.tile_pool(name="ps", space="PSUM", bufs=1))

        # ---- HAM warmup + exp-table preload: no DMA dependencies ----
        warm_sb = const.tile([128, SC], F8, name="warm_sb")
        nc.gpsimd.memset(warm_sb[:], 0)
        dummy_sb = const.tile([1, 1], F32, name="dummy_sb")
        nc.scalar.activation(
            out=dummy_sb[:],
            in_=warm_sb[0:1, 0:4].bitcast(F32),
            func=mybir.ActivationFunctionType.Exp,
            scale=1.0,
        )
        for i in range(13):
            ps_warm = ps.tile([128, SC], F32, tag="pk", bufs=2, name=f"ps_warm{i}")
            nc.tensor.matmul(
                ps_warm[:], warm_sb[:, 0:128], warm_sb[:], start=True, stop=True
            )

        # ---- input DMAs: scalar queue only gets early never-blocking
        # issues; the bulk streams on sync, ordered by first use ----
        hot_sb = const.tile([128, HOT_B], mybir.dt.uint8, name="hot_sb")
        cold_sb = const.tile([128, COLD_B], mybir.dt.uint8, name="cold_sb")
        f8_00 = big.tile([128, 1, 2, SC], F8, name="f8_00")
        f8_01 = big.tile([128, 1, 2, SC], F8, name="f8_01")
        f8_0b = big.tile([128, 2, 2, SC], F8, name="f8_0b")
        seqf8 = [None] + [
            big.tile([128, DR, 2, SC], F8, name=f"seqf8_{j}") for j in range(1, NSJ)
        ]
        seqbh = [
            big.tile([128, DCH // 2, SC], BF, name=f"seqb_{j}_{h}")
            for j in range(2)
            for h in range(2)
        ]
        seqbf = [big.tile([128, DCH, SC], BF, name=f"seqb_{j}") for j in (2, 3)]

        nc.scalar.dma_start(out=f8_00[:], in_=seqf8_d[:, 0, 0:1, :, :])
        nc.sync.dma_start(out=hot_sb[:], in_=ch_d.ap())
        nc.scalar.dma_start(out=f8_01[:], in_=seqf8_d[:, 0, 1:2, :, :])
        nc.sync.dma_start(out=f8_0b[:], in_=seqf8_d[:, 0, 2:4, :, :])
        nc.scalar.dma_start(out=seqf8[1][:], in_=seqf8_d[:, 1])
        nc.sync.dma_start(out=seqf8[2][:], in_=seqf8_d[:, 2])
        nc.sync.dma_start(out=seqf8[3][:], in_=seqf8_d[:, 3])
        nc.scalar.dma_start(out=cold_sb[:], in_=cc_d.ap())
        nc.scalar.dma_start(out=seqbh[0][:], in_=seqb_d[:, 0, 0:4, :])
        nc.sync.dma_start(out=seqbh[1][:], in_=seqb_d[:, 0, 4:8, :])
        nc.scalar.dma_start(out=seqbh[2][:], in_=seqb_d[:, 1, 0:4, :])
        nc.sync.dma_start(out=seqbh[3][:], in_=seqb_d[:, 1, 4:8, :])
        nc.sync.dma_start(out=seqbf[0][:], in_=seqb_d[:, 2])
        nc.sync.dma_start(out=seqbf[1][:], in_=seqb_d[:, 3])

        def seqb_half(u):
            sj, h = divmod(u, 2)
            if sj < 2:
                return seqbh[u][:, :, :]
            return seqbf[sj - 2][:, 4 * h : 4 * h + 4, :]

        wkq_sb = hot_sb[:, 0:1024].bitcast(F8).rearrange(
            "p (c i f) -> p c i f", c=DR, i=2
        )
        wqk_sb = hot_sb[:, 1024:2048].bitcast(F8).rearrange(
            "p (c i f) -> p c i f", c=DR, i=2
        )
        misc_sb = hot_sb[:, 2048:2124].bitcast(F32)
        wv_sb = cold_sb[:, 0:1024].bitcast(BF).rearrange("p (c f) -> p c f", c=DCH)
        identb_sb = cold_sb[:, 1024:1280].bitcast(BF)
        identh_sb = cold_sb[:, 1280:1410].bitcast(F16)

        kqT = [big.tile([128, SC], BF, name=f"kqT_{j}") for j in range(NSJ)]
        kq2T = [big.tile([128, SC], BF, name=f"kq2T_{j}") for j in range(NSJ)]
        vT = [big.tile([F, SC], BF, name=f"vT_{j}") for j in range(NSJ)]
        v_sbs = [big.tile([128, 4, F + 1], BF, name=f"v_sb{j}") for j in range(NSJ)]
        out_sbs = [big.tile([128, 4, F], F32, name=f"out_sb{q}") for q in range(4)]
        out_r = out_d.ap().rearrange("(c p) f -> p c f", p=128)

        bkq_ap = misc_sb[:, 0:1]
        bqk_ap = misc_sb[:, 1:2]
        bv_ap = misc_sb[0:F, 2:3]
        mask01 = misc_sb[:, 3:]

        for j in range(NSJ):
            nc.gpsimd.tensor_copy(v_sbs[j][:, :, F], mask01[:, 4 * j : 4 * j + 4])

        ctx_tiles = {}
        pending_ctx = []

        def emit_ctx(qc, p, wA, wB):
            ctx_ps = ctx_tiles[qc]
            ka, kb = 2 * p, 2 * p + 1
            nc.tensor.matmul(
                ctx_ps[:],
                v_sbs[ka // 4][:, ka % 4, :],
                wA,
                start=(p == 0),
                stop=False,
            )
            nc.tensor.matmul(
                ctx_ps[:],
                v_sbs[kb // 4][:, kb % 4, :],
                wB,
                start=False,
                stop=(p == KCH // 2 - 1),
            )

        def pop_ctx(n):
            for _ in range(min(n, len(pending_ctx))):
                qc, p, wA, wB = pending_ctx.pop(0)
                emit_ctx(qc, p, wA, wB)
                if p == KCH // 2 - 1:
                    finalize(qc)

        def pair_block(qc, p):
            if qc not in ctx_tiles:
                ctx_tiles[qc] = ps.tile(
                    [F + 1, SC], F32, tag="ctx", bufs=2, name=f"ctx_ps{qc}"
                )
            ka, kb = 2 * p, 2 * p + 1
            ps_pair = ps.tile(
                [128, 2 * SC], F32, tag="pair", bufs=2, name=f"ps_pair_{qc}_{p}"
            )
            nc.tensor.matmul(
                ps_pair[:, 0:SC],
                kqT[ka // 4][0:F, (ka % 4) * 128 : (ka % 4 + 1) * 128],
                kq2T[qc][0:F, :],
                start=True,
                stop=True,
            )
            nc.tensor.matmul(
                ps_pair[:, SC : 2 * SC],
                kq2T[kb // 4][64:128, (kb % 4) * 128 : (kb % 4 + 1) * 128],
                kqT[qc][64:128, :],
                start=True,
                stop=True,
            )
            if qc == 3 and p == 7:
                # the very last weights: two half-exps so the final ctx
                # matmuls start ~0.7us earlier (shorter serial tail)
                eA = sbw.tile([128, SC], BF, tag="expq", bufs=16, name="expq_37a")
                eB = sbw.tile([128, SC], BF, tag="expq", bufs=16, name="expq_37b")
                for h, e in enumerate((eA, eB)):
                    nc.scalar.activation(
                        out=e[:],
                        in_=ps_pair[:, h * SC : (h + 1) * SC],
                        func=mybir.ActivationFunctionType.Exp,
                        scale=SCALE,
                    )
                pending_ctx.append((qc, p, eA[:], eB[:]))
            else:
                expq = sbw.tile(
                    [128, 2 * SC], BF, tag="expq", bufs=16, name=f"expq_{qc}_{p}"
                )
                nc.scalar.activation(
                    out=expq[:],
                    in_=ps_pair[:],
                    func=mybir.ActivationFunctionType.Exp,
                    scale=SCALE,
                )
                pending_ctx.append((qc, p, expq[:, 0:SC], expq[:, SC : 2 * SC]))

        vps = {}

        def v_unit(u):
            sj, h = divmod(u, 2)
            if h == 0:
                vps[sj] = ps.tile([F, SC], F32, tag="pk", bufs=2, name=f"ps_v{sj}")
            for c in range(4 * h, 4 * h + 4):
                nc.tensor.matmul(
                    vps[sj][:],
                    wv_sb[:, c, :],
                    seqb_half(u)[:, c - 4 * h, :],
                    start=(c == 0),
                    stop=(c == DCH - 1),
                )
            if h == 1:
                nc.vector.tensor_scalar_add(vT[sj][:], vps[sj][:], bv_ap)
                for i in range(4):
                    t = 4 * sj + i
                    vtp = ps.tile([128, F], BF, tag="pk", bufs=2, name=f"vtp{t}")
                    nc.tensor.transpose(
                        vtp[:],
                        vT[sj][:, i * 128 : (i + 1) * 128],
                        identb_sb[0:F, 0:F],
                    )
                    nc.vector.tensor_scalar_mul(
                        v_sbs[sj][:, i, 0:F], vtp[:], mask01[:, t : t + 1]
                    )

        def finalize(qc):
            ctx_ps = ctx_tiles.pop(qc)
            ctxTq = sbw.tile([F + 1, SC], F16, tag="ctxTq", bufs=2, name=f"ctxTq{qc}")
            nc.vector.tensor_copy(ctxTq[:], ctx_ps[:])
            for i in range(SC // 128):
                t = qc * 4 + i
                ctp = ps.tile([128, F + 1], F16, tag="pk", bufs=2, name=f"ctp{t}")
                nc.tensor.transpose(
                    ctp[:],
                    ctxTq[:, i * 128 : (i + 1) * 128],
                    identh_sb[0 : F + 1, 0 : F + 1],
                )
                rec = sbw.tile([128, 1], F32, tag="rec", bufs=4, name=f"rec{t}")
                nc.vector.reciprocal(rec[:], ctp[:, F : F + 1])
                nc.vector.tensor_scalar_mul(
                    out_sbs[qc][:, i, :], ctp[:, 0:F], rec[:]
                )
                if qc == 3 and i == 1:
                    nc.sync.dma_start(
                        out=out_r[:, 12:14, :], in_=out_sbs[3][:, 0:2, :]
                    )
            if qc == 3:
                nc.sync.dma_start(out=out_r[:, 14:16, :], in_=out_sbs[3][:, 2:4, :])
            else:
                nc.sync.dma_start(
                    out=out_r[:, qc * 4 : (qc + 1) * 4, :],
                    in_=out_sbs[qc][:],
                )

        # ---- Phase A: K/Q projections with q-chunk 0's pair blocks ----
        for sj in range(NSJ):
            ps_kq = ps.tile([128, SC], F32, tag="pk", bufs=2, name=f"ps_kq{sj}")
            ps_kq2 = ps.tile([128, SC], F32, tag="pk", bufs=2, name=f"ps_kq2_{sj}")
            for c in range(DR):
                if sj == 0:
                    if c < 2:
                        rhs = (f8_00 if c == 0 else f8_01)[:, 0, :, :]
                    else:
                        rhs = f8_0b[:, c - 2, :, :]
                else:
                    rhs = seqf8[sj][:, c, :, :]
                st = dict(start=(c == 0), stop=(c == DR - 1))
                nc.tensor.matmul(
                    ps_kq[:], wkq_sb[:, c, :, :], rhs,
                    perf_mode=mybir.MatmulPerfMode.DoubleRow, **st
                )
                nc.tensor.matmul(
                    ps_kq2[:], wqk_sb[:, c, :, :], rhs,
                    perf_mode=mybir.MatmulPerfMode.DoubleRow, **st
                )
            nc.vector.tensor_scalar_add(kqT[sj][:], ps_kq[:], bkq_ap)
            nc.vector.tensor_scalar_add(kq2T[sj][:], ps_kq2[:], bqk_ap)
            pair_block(0, 2 * sj)
            pair_block(0, 2 * sj + 1)

        # ---- Phases B/C/D: qc 1..3 pair blocks, exp-paced ----
        for p in range(KCH // 2):  # qc = 1
            pair_block(1, p)
            if p < 6:
                v_unit(p)
            else:
                pop_ctx(2)
        for p in range(KCH // 2):  # qc = 2
            pair_block(2, p)
            if p < 2:
                v_unit(6 + p)
                pop_ctx(1)
            else:
                pop_ctx(2)
        for p in range(KCH // 2):  # qc = 3
            pair_block(3, p)
            pop_ctx(2)
        pop_ctx(len(pending_ctx))
        pop_ctx(len(pending_ctx))


_NC_CACHE = None


def _get_nc():
    global _NC_CACHE
    if _NC_CACHE is None:
        nc = bacc.Bacc("TRN2", target_bir_lowering=False, debug=False)
        _emit(nc)
        _NC_CACHE = nc
    return _NC_CACHE


def make_in_maps(seq, mask, Wq, bq, Wk, bk, Wv, bv):
    bf16 = ml_dtypes.bfloat16
    f16 = np.float16
    f8 = ml_dtypes.float8_e4m3
    seq = np.asarray(seq, dtype=np.float32)
    mask = np.asarray(mask).astype(bool)
    wkq = np.concatenate(
        [np.asarray(Wk, dtype=np.float32), np.asarray(Wq, dtype=np.float32)], axis=1
    )
    wqk = np.concatenate(
        [np.asarray(Wq, dtype=np.float32), np.asarray(Wk, dtype=np.float32)], axis=1
    )
    wkq_h = np.ascontiguousarray(
        (wkq * 32.0).astype(f8).reshape(DR, 128, 2, 128).transpose(1, 0, 2, 3)
    )
    wqk_h = np.ascontiguousarray(
        (wqk * 32.0).astype(f8).reshape(DR, 128, 2, 128).transpose(1, 0, 2, 3)
    )
    wv_h = np.ascontiguousarray(
        np.asarray(Wv, dtype=np.float32).astype(bf16).reshape(DCH, 128, F).transpose(1, 0, 2)
    )
    cold = np.zeros((128, COLD_B), dtype=np.uint8)
    cold[:, 0:1024] = wv_h.reshape(128, 512).view(np.uint8)
    cold[:, 1024:1280] = np.eye(128, dtype=bf16).view(np.uint8)
    ih = np.zeros((128, 65), dtype=f16)
    ih[0:65] = np.eye(65, dtype=f16)
    cold[:, 1280:1410] = ih.view(np.uint8)
    hot = np.zeros((NCORES, 128, HOT_B), dtype=np.uint8)
    hot[:, :, 0:1024] = wkq_h.reshape(128, 1024).view(np.uint8)
    hot[:, :, 1024:2048] = wqk_h.reshape(128, 1024).view(np.uint8)
    in_maps = []
    for b in range(NCORES):
        seqT = np.ascontiguousarray(seq[b].T)
        # fp8, partition-outermost: [p, sj, c, i, t]
        sf8 = np.ascontiguousarray(
            seqT.astype(f8).reshape(DR, 128, 2, NSJ, SC).transpose(1, 3, 0, 2, 4)
        )
        # bf16, partition-outermost: [p, sj, c, t]
        sb16 = np.ascontiguousarray(
            seqT.astype(bf16).reshape(DCH, 128, NSJ, SC).transpose(1, 2, 0, 3)
        )
        misc = np.zeros((128, 3 + KCH), dtype=np.float32)
        misc[0:F, 0] = 32.0 * np.asarray(bk, dtype=np.float32)
        misc[64:128, 0] = 32.0 * np.asarray(bq, dtype=np.float32)
        misc[0:F, 1] = 32.0 * np.asarray(bq, dtype=np.float32)
        misc[64:128, 1] = 32.0 * np.asarray(bk, dtype=np.float32)
        misc[0:F, 2] = np.asarray(bv, dtype=np.float32)
        misc[:, 3:] = np.where(mask[b], np.float32(0.0), np.float32(1.0)).reshape(
            KCH, 128
        ).T
        hot[b, :, 2048:2124] = misc.view(np.uint8)
        in_maps.append(
            {
                "seqf8": sf8,
                "seqb": sb16,
                "ch": hot[b],
                "cc": cold,
            }
        )
    return in_maps


def run(in_maps, trace=False, **kw):
    nc = _get_nc()
    return run_bass_kernel_spmd(
        nc, in_maps, core_ids=list(range(NCORES)), trace=trace, **kw
    )


def kernel(seq, mask, Wq, bq, Wk, bk, Wv, bv):
    in_maps = make_in_maps(seq, mask, Wq, bq, Wk, bk, Wv, bv)
    res = run(in_maps)
    out = np.stack(
        [np.asarray(res.results[i]["out"], dtype=np.float32) for i in range(NCORES)],
        axis=0,
    )
    return out


# revision 33
# speedup vs baseline: 1.0138x; 1.0138x over previous
"""Single attention head (B=8, S=2048, D_IN=1024, D_OUT=64) on 8 TRN2 NeuronCores.

Strategy: pure data-parallel over batch -- core b computes batch element b's
full attention head. No collectives.

v9 -- the v3 baseline schedule (exp-paced, qc-major, deferred ctx pops)
with four independently-verified upgrades:
  - ~4.5us of junk matmuls on a memset tile (no DMA dependency) pull the
    PE HAM clock-gate through the SHORT window to 8/8 while the first
    input DMAs are still in flight (first-chunk DMA latency is ~5us);
    v3's warmups waited on the consts DMA, so all of phase A ran at the
    1.2GHz cold clock.
  - The consts DMA is split hot (wkq/wqk/misc -- everything phase A
    needs) / cold (wv/identities, first needed by the phase-B V units),
    and seqf8[0] lands as c0 / c1 / c2c3 chunks, so the first projection
    starts as early as the DMA pipe allows.  Host layouts are
    partition-outermost so any [sj, c-range] slice is one
    contiguous-per-partition descriptor.  The scalar HWDGE queue gets
    only early never-blocking issues (a credit-blocked issue instruction
    stalls the exp stream behind it in ScalarE's FIFO).
  - finalize runs in fp16 (PE transpose w/ fp16 identity): half the f32
    transpose cycles and 2x DVE rate on the psum drain.  |num| <~ 1e3,
    den ~ 1.7e3: well inside fp16 range, 4.9e-4 rel quantization.
  - The very last exp (qc3, pair 7) is split into two half-tiles and
    qc3's output DMA into two chunks, shortening the serial tail after
    the final exp by ~1us.
Per-core dataflow otherwise identical to v3 (see its docstring): fp8
DoubleRow K/Q projections with x32-scaled stacked weights, score pairs
co-run on disjoint PE row groups, one exp per [128,1024] pair tile, mask
applied via zeroed V rows + keep-column denominator, ctx accumulated as
ctxT[65, q] with deferred pops as PE filler.
"""

import numpy as np
import ml_dtypes

import concourse.bass as bass  # noqa: F401
import concourse.mybir as mybir
import concourse.tile as tile
from concourse import bacc
from concourse.bass_utils import run_bass_kernel_spmd

B, S, D, F = 8, 2048, 1024, 64
NCORES = 8
BF = mybir.dt.bfloat16
F16 = mybir.dt.float16
F8 = mybir.dt.float8e4
F32 = mybir.dt.float32
SCALE = 1.0 / (1024.0 * float(np.sqrt(np.float32(S))))
SC = 512
NSJ = S // SC
KCH = S // 128
DCH = D // 128
DR = D // 256
HOT_B = 2124  # wkq 1024 | wqk 1024 | misc 76
COLD_B = 1410  # wv 1024 | identb 256 | identh 130


def _emit(nc):
    seqf8_d = nc.declare_dram_parameter("seqf8", [128, NSJ, DR, 2, SC], F8, isOutput=False)
    seqb_d = nc.declare_dram_parameter("seqb", [128, NSJ, DCH, SC], BF, isOutput=False)
    ch_d = nc.declare_dram_parameter("ch", [128, HOT_B], mybir.dt.uint8, isOutput=False)
    cc_d = nc.declare_dram_parameter("cc", [128, COLD_B], mybir.dt.uint8, isOutput=False)
    out_d = nc.declare_dram_parameter("out", [S, F], F32, isOutput=True)

    with tile.TileContext(nc) as tc:
        _body(nc, tc, seqf8_d, seqb_d, ch_d, cc_d, out_d)
    nc.compile()


def _body(nc, tc, seqf8_d, seqb_d, ch_d, cc_d, out_d):
    from contextlib import ExitStack

    with ExitStack() as ctx:
        const = ctx.enter_context(tc.tile_pool(name="const", bufs=1))
        big = ctx.enter_context(tc.tile_pool(name="big", bufs=1))
        sbw = ctx.enter_context(tc.tile_pool(name="sbw", bufs=1))
        ps = ctx.enter_context(tc.tile_pool(name="ps", space="PSUM", bufs=1))

        # ---- HAM warmup + exp-table preload: no DMA dependencies ----
        warm_sb = const.tile([128, SC], F8, name="warm_sb")
        nc.gpsimd.memset(warm_sb[:], 0)
        dummy_sb = const.tile([1, 1], F32, name="dummy_sb")
        nc.scalar.activation(
            out=dummy_sb[:],
            in_=warm_sb[0:1, 0:4].bitcast(F32),
            func=mybir.ActivationFunctionType.Exp,
            scale=1.0,
        )
        for i in range(13):
            ps_warm = ps.tile([128, SC], F32, tag="pk", bufs=2, name=f"ps_warm{i}")
            nc.tensor.matmul(
                ps_warm[:], warm_sb[:, 0:128], warm_sb[:], start=True, stop=True
            )

        # ---- input DMAs: scalar queue only gets early never-blocking
        # issues; the bulk streams on sync, ordered by first use ----
        hot_sb = const.tile([128, HOT_B], mybir.dt.uint8, name="hot_sb")
        cold_sb = const.tile([128, COLD_B], mybir.dt.uint8, name="cold_sb")
        f8_00 = big.tile([128, 1, 2, SC], F8, name="f8_00")
        f8_01 = big.tile([128, 1, 2, SC], F8, name="f8_01")
        f8_0b = big.tile([128, 2, 2, SC], F8, name="f8_0b")
        seqf8 = [None] + [
            big.tile([128, DR, 2, SC], F8, name=f"seqf8_{j}") for j in range(1, NSJ)
        ]
        seqbh = [
            big.tile([128, DCH // 2, SC], BF, name=f"seqb_{j}_{h}")
            for j in range(2)
            for h in range(2)
        ]
        seqbf = [big.tile([128, DCH, SC], BF, name=f"seqb_{j}") for j in (2, 3)]

        nc.scalar.dma_start(out=f8_00[:], in_=seqf8_d[:, 0, 0:1, :, :])
        nc.sync.dma_start(out=hot_sb[:], in_=ch_d.ap())
        nc.scalar.dma_start(out=f8_01[:], in_=seqf8_d[:, 0, 1:2, :, :])
        nc.sync.dma_start(out=f8_0b[:], in_=seqf8_d[:, 0, 2:4, :, :])
        nc.scalar.dma_start(out=seqf8[1][:], in_=seqf8_d[:, 1])
        nc.sync.dma_start(out=seqf8[2][:], in_=seqf8_d[:, 2])
        nc.sync.dma_start(out=seqf8[3][:], in_=seqf8_d[:, 3])
        nc.scalar.dma_start(out=cold_sb[:], in_=cc_d.ap())
        nc.scalar.dma_start(out=seqbh[0][:], in_=seqb_d[:, 0, 0:4, :])
        nc.sync.dma_start(out=seqbh[1][:], in_=seqb_d[:, 0, 4:8, :])
        nc.scalar.dma_start(out=seqbh[2][:], in_=seqb_d[:, 1, 0:4, :])
        nc.sync.dma_start(out=seqbh[3][:], in_=seqb_d[:, 1, 4:8, :])
        nc.sync.dma_start(out=seqbf[0][:], in_=seqb_d[:, 2])
        nc.sync.dma_start(out=seqbf[1][:], in_=seqb_d[:, 3])

        def seqb_half(u):
            sj, h = divmod(u, 2)
            if sj < 2:
                return seqbh[u][:, :, :]
            return seqbf[sj - 2][:, 4 * h : 4 * h + 4, :]

        wkq_sb = hot_sb[:, 0:1024].bitcast(F8).rearrange(
            "p (c i f) -> p c i f", c=DR, i=2
        )
        wqk_sb = hot_sb[:, 1024:2048].bitcast(F8).rearrange(
            "p (c i f) -> p c i f", c=DR, i=2
        )
        misc_sb = hot_sb[:, 2048:2124].bitcast(F32)
        wv_sb = cold_sb[:, 0:1024].bitcast(BF).rearrange("p (c f) -> p c f", c=DCH)
        identb_sb = cold_sb[:, 1024:1280].bitcast(BF)
        identh_sb = cold_sb[:, 1280:1410].bitcast(F16)

        kqT = [big.tile([128, SC], BF, name=f"kqT_{j}") for j in range(NSJ)]
        kq2T = [big.tile([128, SC], BF, name=f"kq2T_{j}") for j in range(NSJ)]
        vT = [big.tile([F, SC], BF, name=f"vT_{j}") for j in range(NSJ)]
        v_sbs = [big.tile([128, 4, F + 1], BF, name=f"v_sb{j}") for j in range(NSJ)]
        out_sbs = [big.tile([128, 4, F], F32, name=f"out_sb{q}") for q in range(4)]
        out_r = out_d.ap().rearrange("(c p) f -> p c f", p=128)

        bkq_ap = misc_sb[:, 0:1]
        bqk_ap = misc_sb[:, 1:2]
        bv_ap = misc_sb[0:F, 2:3]
        mask01 = misc_sb[:, 3:]

        for j in range(NSJ):
            nc.gpsimd.tensor_copy(v_sbs[j][:, :, F], mask01[:, 4 * j : 4 * j + 4])

        ctx_tiles = {}
        pending_ctx = []

        def emit_ctx(qc, p, wA, wB):
            ctx_ps = ctx_tiles[qc]
            ka, kb = 2 * p, 2 * p + 1
            nc.tensor.matmul(
                ctx_ps[:],
                v_sbs[ka // 4][:, ka % 4, :],
                wA,
                start=(p == 0),
                stop=False,
            )
            nc.tensor.matmul(
                ctx_ps[:],
                v_sbs[kb // 4][:, kb % 4, :],
                wB,
                start=False,
                stop=(p == KCH // 2 - 1),
            )

        def pop_ctx(n):
            for _ in range(min(n, len(pending_ctx))):
                qc, p, wA, wB = pending_ctx.pop(0)
                emit_ctx(qc, p, wA, wB)
                if p == KCH // 2 - 1:
                    finalize(qc)

        def pair_block(qc, p):
            if qc not in ctx_tiles:
                ctx_tiles[qc] = ps.tile(
                    [F + 1, SC], F32, tag="ctx", bufs=2, name=f"ctx_ps{qc}"
                )
            ka, kb = 2 * p, 2 * p + 1
            ps_pair = ps.tile(
                [128, 2 * SC], F32, tag="pair", bufs=2, name=f"ps_pair_{qc}_{p}"
            )
            nc.tensor.matmul(
                ps_pair[:, 0:SC],
                kqT[ka // 4][0:F, (ka % 4) * 128 : (ka % 4 + 1) * 128],
                kq2T[qc][0:F, :],
                start=True,
                stop=True,
            )
            nc.tensor.matmul(
                ps_pair[:, SC : 2 * SC],
                kq2T[kb // 4][64:128, (kb % 4) * 128 : (kb % 4 + 1) * 128],
                kqT[qc][64:128, :],
                start=True,
                stop=True,
            )
            if qc == 3 and p == 7:
                # the very last weights: two half-exps so the final ctx
                # matmuls start ~0.7us earlier (shorter serial tail)
                eA = sbw.tile([128, SC], BF, tag="expq", bufs=16, name="expq_37a")
                eB = sbw.tile([128, SC], BF, tag="expq", bufs=16, name="expq_37b")
                for h, e in enumerate((eA, eB)):
                    nc.scalar.activation(
                        out=e[:],
                        in_=ps_pair[:, h * SC : (h + 1) * SC],
                        func=mybir.ActivationFunctionType.Exp,
                        scale=SCALE,
                    )
                pending_ctx.append((qc, p, eA[:], eB[:]))
            else:
                expq = sbw.tile(
                    [128, 2 * SC], BF, tag="expq", bufs=16, name=f"expq_{qc}_{p}"
                )
                nc.scalar.activation(
                    out=expq[:],
                    in_=ps_pair[:],
                    func=mybir.ActivationFunctionType.Exp,
                    scale=SCALE,
                )
                pending_ctx.append((qc, p, expq[:, 0:SC], expq[:, SC : 2 * SC]))

        vps = {}

        def v_unit(u):
            sj, h = divmod(u, 2)
            if h == 0:
                vps[sj] = ps.tile([F, SC], F32, tag="pk", bufs=2, name=f"ps_v{sj}")
            for c in range(4 * h, 4 * h + 4):
                nc.tensor.matmul(
                    vps[sj][:],
                    wv_sb[:, c, :],
                    seqb_half(u)[:, c - 4 * h, :],
                    start=(c == 0),
                    stop=(c == DCH - 1),
                )
            if h == 1:
                nc.vector.tensor_scalar_add(vT[sj][:], vps[sj][:], bv_ap)
                for i in range(4):
                    t = 4 * sj + i
                    vtp = ps.tile([128, F], BF, tag="pk", bufs=2, name=f"vtp{t}")
                    nc.tensor.transpose(
                        vtp[:],
                        vT[sj][:, i * 128 : (i + 1) * 128],
                        identb_sb[0:F, 0:F],
                    )
                    nc.vector.tensor_scalar_mul(
                        v_sbs[sj][:, i, 0:F], vtp[:], mask01[:, t : t + 1]
                    )

        def finalize(qc):
            ctx_ps = ctx_tiles.pop(qc)
            ctxTq = sbw.tile([F + 1, SC], F16, tag="ctxTq", bufs=2, name=f"ctxTq{qc}")
            nc.vector.tensor_copy(ctxTq[:], ctx_ps[:])
            for i in range(SC // 128):
                t = qc * 4 + i
                ctp = ps.tile([128, F + 1], F16, tag="pk", bufs=2, name=f"ctp{t}")
                nc.tensor.transpose(
                    ctp[:],
                    ctxTq[:, i * 128 : (i + 1) * 128],
                    identh_sb[0 : F + 1, 0 : F + 1],
                )
                rec = sbw.tile([128, 1], F32, tag="rec", bufs=4, name=f"rec{t}")
                nc.vector.reciprocal(rec[:], ctp[:, F : F + 1])
                nc.vector.tensor_scalar_mul(
                    out_sbs[qc][:, i, :], ctp[:, 0:F], rec[:]
                )
                if qc == 3 and i >= 1:
                    # per-block stores: the final transfer on the critical
                    # tail is only 64KB
                    if i == 1:
                        nc.sync.dma_start(
                            out=out_r[:, 12:14, :], in_=out_sbs[3][:, 0:2, :]
                        )
                    else:
                        nc.sync.dma_start(
                            out=out_r[:, 12 + i : 13 + i, :],
                            in_=out_sbs[3][:, i : i + 1, :],
                        )
            if qc == 3:
                pass
            else:
                nc.sync.dma_start(
                    out=out_r[:, qc * 4 : (qc + 1) * 4, :],
                    in_=out_sbs[qc][:],
                )

        # ---- Phase A: K/Q projections with q-chunk 0's pair blocks ----
        for sj in range(NSJ):
            ps_kq = ps.tile([128, SC], F32, tag="pk", bufs=2, name=f"ps_kq{sj}")
            ps_kq2 = ps.tile([128, SC], F32, tag="pk", bufs=2, name=f"ps_kq2_{sj}")
            for c in range(DR):
                if sj == 0:
                    if c < 2:
                        rhs = (f8_00 if c == 0 else f8_01)[:, 0, :, :]
                    else:
                        rhs = f8_0b[:, c - 2, :, :]
                else:
                    rhs = seqf8[sj][:, c, :, :]
                st = dict(start=(c == 0), stop=(c == DR - 1))
                nc.tensor.matmul(
                    ps_kq[:], wkq_sb[:, c, :, :], rhs,
                    perf_mode=mybir.MatmulPerfMode.DoubleRow, **st
                )
                nc.tensor.matmul(
                    ps_kq2[:], wqk_sb[:, c, :, :], rhs,
                    perf_mode=mybir.MatmulPerfMode.DoubleRow, **st
                )
            nc.vector.tensor_scalar_add(kqT[sj][:], ps_kq[:], bkq_ap)
            nc.vector.tensor_scalar_add(kq2T[sj][:], ps_kq2[:], bqk_ap)
            pair_block(0, 2 * sj)
            pair_block(0, 2 * sj + 1)

        # ---- Phases B/C/D: qc 1..3 pair blocks, exp-paced ----
        for p in range(KCH // 2):  # qc = 1
            pair_block(1, p)
            if p < 6:
                v_unit(p)
            else:
                pop_ctx(2)
        for p in range(KCH // 2):  # qc = 2
            pair_block(2, p)
            if p < 2:
                v_unit(6 + p)
                pop_ctx(1)
            else:
                pop_ctx(2)
        for p in range(KCH // 2):  # qc = 3
            pair_block(3, p)
            pop_ctx(2)
        pop_ctx(len(pending_ctx))
        pop_ctx(len(pending_ctx))


_NC_CACHE = None


def _get_nc():
    global _NC_CACHE
    if _NC_CACHE is None:
        nc = bacc.Bacc("TRN2", target_bir_lowering=False, debug=False)
        _emit(nc)
        _NC_CACHE = nc
    return _NC_CACHE


def make_in_maps(seq, mask, Wq, bq, Wk, bk, Wv, bv):
    bf16 = ml_dtypes.bfloat16
    f16 = np.float16
    f8 = ml_dtypes.float8_e4m3
    seq = np.asarray(seq, dtype=np.float32)
    mask = np.asarray(mask).astype(bool)
    wkq = np.concatenate(
        [np.asarray(Wk, dtype=np.float32), np.asarray(Wq, dtype=np.float32)], axis=1
    )
    wqk = np.concatenate(
        [np.asarray(Wq, dtype=np.float32), np.asarray(Wk, dtype=np.float32)], axis=1
    )
    wkq_h = np.ascontiguousarray(
        (wkq * 32.0).astype(f8).reshape(DR, 128, 2, 128).transpose(1, 0, 2, 3)
    )
    wqk_h = np.ascontiguousarray(
        (wqk * 32.0).astype(f8).reshape(DR, 128, 2, 128).transpose(1, 0, 2, 3)
    )
    wv_h = np.ascontiguousarray(
        np.asarray(Wv, dtype=np.float32).astype(bf16).reshape(DCH, 128, F).transpose(1, 0, 2)
    )
    cold = np.zeros((128, COLD_B), dtype=np.uint8)
    cold[:, 0:1024] = wv_h.reshape(128, 512).view(np.uint8)
    cold[:, 1024:1280] = np.eye(128, dtype=bf16).view(np.uint8)
    ih = np.zeros((128, 65), dtype=f16)
    ih[0:65] = np.eye(65, dtype=f16)
    cold[:, 1280:1410] = ih.view(np.uint8)
    hot = np.zeros((NCORES, 128, HOT_B), dtype=np.uint8)
    hot[:, :, 0:1024] = wkq_h.reshape(128, 1024).view(np.uint8)
    hot[:, :, 1024:2048] = wqk_h.reshape(128, 1024).view(np.uint8)
    in_maps = []
    for b in range(NCORES):
        seqT = np.ascontiguousarray(seq[b].T)
        # fp8, partition-outermost: [p, sj, c, i, t]
        sf8 = np.ascontiguousarray(
            seqT.astype(f8).reshape(DR, 128, 2, NSJ, SC).transpose(1, 3, 0, 2, 4)
        )
        # bf16, partition-outermost: [p, sj, c, t]
        sb16 = np.ascontiguousarray(
            seqT.astype(bf16).reshape(DCH, 128, NSJ, SC).transpose(1, 2, 0, 3)
        )
        misc = np.zeros((128, 3 + KCH), dtype=np.float32)
        misc[0:F, 0] = 32.0 * np.asarray(bk, dtype=np.float32)
        misc[64:128, 0] = 32.0 * np.asarray(bq, dtype=np.float32)
        misc[0:F, 1] = 32.0 * np.asarray(bq, dtype=np.float32)
        misc[64:128, 1] = 32.0 * np.asarray(bk, dtype=np.float32)
        misc[0:F, 2] = np.asarray(bv, dtype=np.float32)
        misc[:, 3:] = np.where(mask[b], np.float32(0.0), np.float32(1.0)).reshape(
            KCH, 128
        ).T
        hot[b, :, 2048:2124] = misc.view(np.uint8)
        in_maps.append(
            {
                "seqf8": sf8,
                "seqb": sb16,
                "ch": hot[b],
                "cc": cold,
            }
        )
    return in_maps


def run(in_maps, trace=False, **kw):
    nc = _get_nc()
    return run_bass_kernel_spmd(
        nc, in_maps, core_ids=list(range(NCORES)), trace=trace, **kw
    )


def kernel(seq, mask, Wq, bq, Wk, bk, Wv, bv):
    in_maps = make_in_maps(seq, mask, Wq, bq, Wk, bk, Wv, bv)
    res = run(in_maps)
    out = np.stack(
        [np.asarray(res.results[i]["out"], dtype=np.float32) for i in range(NCORES)],
        axis=0,
    )
    return out
